# revision 23
# baseline (speedup 1.0000x reference)
"""Causal self-attention (B=2, T=2048, E=1024, H=16) on 8 trn2 NeuronCores.

Sharding: tensor-parallel over heads - core c owns heads {2c, 2c+1}.
Each core computes qkv for its 2 heads, causal attention, and a partial
[B, T, E] output projection over its 128 columns of w_proj; the host
sums the 8 partials.

v2 design (ACT/PE balanced, ~all engines overlapped):
  - attention runs on 512-wide q windows; per k-tile the two heads'
    scores matmuls are a ROW-TILED CONCURRENT PAIR (K=64 contraction,
    h0 on PE rows 0-63, h1 on rows 64-127, auto tile_position from
    base_partition) writing one [128, 2, 512] PSUM tile (adjacent
    banks), so both heads' scores cost one matmul's wall time;
  - exp for BOTH heads is ONE ACTIVATE instruction (strided over the
    2-bank psum tile) - the scalar engine is the attention bottleneck
    (1 elem/cycle/lane), so instruction count is minimized;
  - qkv projection of batch b+1 and the output projection of batch b-1
    are emitted to interleave into the ACT-bound attention span, so the
    tensor engine never idles long enough to drop the HAM clock gate;
  - V^T tiles are produced by the DMA xbar transpose engine (not the
    PE) straight into the PV stationary layout;
  - softmax 1/l: l rows for both heads are contiguous in the [65,2,512]
    PV psum tile; one copy -> DMA-reshape to [128,8] -> exact DVE
    reciprocal -> DMA back -> gpsimd partition-broadcast -> 2 muls.

Matmul operands are bf16 (PSUM accumulation fp32); the softmax scale
1/8 is folded into w_q on the host.
"""

import numpy as np
import ml_dtypes
from contextlib import ExitStack

import concourse.bass as bass
import concourse.mybir as mybir
import concourse.tile as tile
from concourse import bacc
from concourse.bass_utils import run_bass_kernel_spmd

B, T, E, H, D = 2, 2048, 1024, 16, 64
NCORES = 8
HPC = H // NCORES          # heads per core = 2
JC = HPC * D               # local out-projection columns per core = 128
W = 512                    # q window (one PSUM bank of fp32)
KT = 128                   # k tile (matmul M limit)
NW = T // W                # windows per batch = 4
NKT = T // KT              # k tiles per batch = 16
NCH = 8                    # E/128 contraction chunks

BF16 = mybir.dt.bfloat16
FP32 = mybir.dt.float32
NPBF = ml_dtypes.bfloat16
EXP = mybir.ActivationFunctionType.Exp
CPY = mybir.ActivationFunctionType.Copy

_NC_CACHE = []


def _build_nc():
    nc = bacc.Bacc(None, target_bir_lowering=False)

    xT = nc.dram_tensor("xT", [E, B, T], BF16, kind="ExternalInput")
    wqkvT = nc.dram_tensor("wqkvT", [E, 3 * JC], BF16, kind="ExternalInput")
    wpT = nc.dram_tensor("wpT", [JC, E], BF16, kind="ExternalInput")
    outp = nc.dram_tensor("outp", [B, T, E], FP32, kind="ExternalOutput")

    with tile.TileContext(nc) as tc, ExitStack() as ctx:
        const_pool = ctx.enter_context(tc.tile_pool(name="const", bufs=1))
        w_pool = ctx.enter_context(tc.tile_pool(name="w", bufs=1))
        xt_pool = ctx.enter_context(tc.tile_pool(name="xt", bufs=1))
        qk_pool = ctx.enter_context(tc.tile_pool(name="qk", bufs=1))
        va_pool = ctx.enter_context(tc.tile_pool(name="va", bufs=1))
        vtmp_pool = ctx.enter_context(tc.tile_pool(name="vtmp", bufs=2))
        probs_pool = ctx.enter_context(tc.tile_pool(name="probs", bufs=8))
        outT_pool = ctx.enter_context(tc.tile_pool(name="outT", bufs=1))
        norm_pool = ctx.enter_context(tc.tile_pool(name="norm", bufs=2))
        st_pool = ctx.enter_context(tc.tile_pool(name="st", bufs=3))
        # PSUM: 8 banks = scp 2x2 + ops 2x1x... ops is [65,2,W] = 2 banks
        # single-buffered + mmbuf 2x1
        scp_pool = ctx.enter_context(tc.tile_pool(name="scp", bufs=2, space="PSUM"))
        ops_pool = ctx.enter_context(tc.tile_pool(name="ops", bufs=1, space="PSUM"))
        mm_pool = ctx.enter_context(tc.tile_pool(name="mmbuf", bufs=2, space="PSUM"))

        # --- constants -------------------------------------------------
        # warmup: pull the exp ACT_TABLE_LOAD (~2.7us) into the prologue
        wrm = const_pool.tile([1, 8], FP32, tag="wrm")
        nc.gpsimd.memset(wrm[:], 0.0)
        wrm2 = const_pool.tile([1, 8], BF16, tag="wrm2")
        nc.scalar.activation(wrm2[:], wrm[:], EXP)

        # mask2[p, h, j] = 1 iff j >= p (causal band for a diagonal tile),
        # same band replicated for both heads so one DVE mul covers both.
        mask2 = const_pool.tile([128, HPC, KT], BF16)
        nc.gpsimd.memset(mask2[:], 1.0)
        for h in range(HPC):
            nc.gpsimd.affine_select(
                out=mask2[:, h, :],
                in_=mask2[:, h, :],
                compare_op=mybir.AluOpType.is_ge,
                fill=0.0,
                base=0,
                channel_multiplier=-1,
                pattern=[[1, KT]],
            )

        # --- weights ---------------------------------------------------
        # weights + whole-row batch-1 x loads on the gpsimd SWDGE queue;
        # the SP hwdge queue is reserved for batch-0 x chunks (just in
        # time) + the xbar transposes so nothing big queues ahead of them
        wq_sb = [
            w_pool.tile([128, 3 * JC], BF16, tag=f"wq{i}", name=f"wq{i}")
            for i in range(NCH)
        ]
        for i in range(NCH):
            nc.gpsimd.dma_start(wq_sb[i][:], wqkvT[i * 128 : (i + 1) * 128, :])
        wp_sb = w_pool.tile([JC, E], BF16, tag="wp")
        nc.gpsimd.dma_start(wp_sb[:], wpT[:])

        xt = [[None] * NCH for _ in range(B)]
        for b in range(B):
            for i in range(NCH):
                xt[b][i] = xt_pool.tile(
                    [128, T], BF16, tag=f"xt{b}_{i}", name=f"xt{b}_{i}"
                )
        for c in range(2):  # chunks 0+1 up front, c+2 prefetched per chunk
            csl0 = slice(c * W, (c + 1) * W)
            for i in range(NCH):
                nc.sync.dma_start(
                    xt[0][i][:, csl0], xT[i * 128 : (i + 1) * 128, 0, csl0]
                )
        for i in range(NCH):
            nc.gpsimd.dma_start(xt[1][i][:], xT[i * 128 : (i + 1) * 128, 1, :])

        # --- per-batch persistent tiles --------------------------------
        QT, KTs, vaug, outTt = [], [], [], []
        for b in range(B):
            QT.append(qk_pool.tile([128, T], BF16, tag=f"QT{b}", name=f"QT{b}"))
            KTs.append(qk_pool.tile([128, T], BF16, tag=f"KT{b}", name=f"KT{b}"))
            va = va_pool.tile(
                [128, NKT, HPC, D + 1], BF16, tag=f"va{b}", name=f"va{b}"
            )
            nc.gpsimd.memset(va[:, :, :, D : D + 1], 1.0)
            vaug.append(va)
            outTt.append(
                outT_pool.tile([128, T], BF16, tag=f"oT{b}", name=f"oT{b}")
            )

        def qkv_chunk(b, c):
            """Project x chunk c into Q^T/K^T cols [cW,(c+1)W) and V^T
            (via DMA xbar transpose) into vaug k-tiles 4c..4c+3."""
            csl = slice(c * W, (c + 1) * W)
            if b == 0 and c + 2 < NW:  # JIT prefetch two chunks ahead,
                # emitted FIRST so it queues ahead of this chunk's xbar
                # transposes on the in-order SP queue
                nsl = slice((c + 2) * W, (c + 3) * W)
                for i in range(NCH):
                    nc.sync.dma_start(
                        xt[0][i][:, nsl], xT[i * 128 : (i + 1) * 128, 0, nsl]
                    )
            for fb in (2, 0, 1):  # V first (transpose latency), K last
                pp = mm_pool.tile([128, W], FP32, tag="mm", name=f"pp{b}{c}{fb}")
                for ct in range(NCH):
                    nc.tensor.matmul(
                        pp[:],
                        wq_sb[ct][:, fb * 128 : (fb + 1) * 128],
                        xt[b][ct][:, csl],
                        start=(ct == 0),
                        stop=(ct == NCH - 1),
                    )
                if fb == 0:
                    nc.vector.tensor_copy(QT[b][:, csl], pp[:])
                elif fb == 1:
                    nc.vector.tensor_copy(KTs[b][:, csl], pp[:])
                else:
                    vtmp = vtmp_pool.tile([128, W], BF16, tag="vt", name=f"vt{b}{c}")
                    nc.vector.tensor_copy(vtmp[:], pp[:])
                    k0 = c * (W // KT)
                    for h in range(HPC):
                        # xbar transpose needs a contiguous dest; stage
                        # then copy into the ones-interleaved vaug slots
                        vstg = vtmp_pool.tile(
                            [128, W // KT, D], BF16, tag="vs", name=f"vs{b}{c}{h}"
                        )
                        nc.sync.dma_start_transpose(
                            vstg[:], vtmp[h * D : (h + 1) * D, :]
                        )
                        nc.vector.tensor_copy(
                            vaug[b][:, k0 : k0 + W // KT, h, 0:D], vstg[:]
                        )

        def scores_exp(b, w, kt):
            q0 = w * W
            koff = kt * KT - q0
            lo = max(0, koff)
            scp = scp_pool.tile([128, HPC, W], FP32, tag="scp", name="scp")
            ksl = slice(kt * KT, (kt + 1) * KT)
            for h in range(HPC):
                nc.tensor.matmul(
                    scp[:, h, lo:W],
                    KTs[b][h * D : (h + 1) * D, ksl],
                    QT[b][h * D : (h + 1) * D, q0 + lo : q0 + W],
                    start=True,
                    stop=True,
                )
            pr = probs_pool.tile([128, HPC, W], BF16, tag="pr", name="pr")
            nc.scalar.activation(pr[:, :, lo:W], scp[:, :, lo:W], EXP)
            if koff >= 0:  # diagonal tile: mask the 128-band per head
                for h in range(HPC):
                    nc.vector.tensor_mul(
                        pr[:, h, lo : lo + KT],
                        pr[:, h, lo : lo + KT],
                        mask2[:, h, :],
                    )
            return pr, lo

        ops_live = {}

        def pv(b, w, kt, pr, lo):
            nkt = 4 * w + 4
            if kt == 0:
                ops_live[b] = ops_pool.tile(
                    [D + 1, HPC, W], FP32, tag="ops", name=f"ops{b}{w}"
                )
            opsb = ops_live[b]
            for h in range(HPC):
                nc.tensor.matmul(
                    opsb[:, h, lo:W],
                    vaug[b][:, kt, h, :],
                    pr[:, h, lo:W],
                    start=(kt == 0),
                    stop=(kt == nkt - 1),
                    skip_group_check=True,
                )
            if kt == nkt - 1:
                finish_window(b, w, opsb)
                if b == 1:  # spread batch-1 proj into its attention span
                    proj_grp(1, w)

        def finish_window(b, w, opsb):
            # Deferred normalize: ONE staging copy releases the psum
            # accumulator; the reciprocal/broadcast/mul chain then runs
            # entirely off the critical path from SBUF.
            sg = norm_pool.tile([D + 1, HPC, W], FP32, tag="sg", name=f"sg{b}{w}")
            nc.vector.tensor_copy(sg[:], opsb[0 : D + 1, :, :])
            lT = norm_pool.tile([128, HPC * W // 128], FP32, tag="lT")
            nc.gpsimd.dma_start(lT[:], sg[D : D + 1, :, :])
            rT = norm_pool.tile([128, HPC * W // 128], FP32, tag="rT")
            nc.vector.reciprocal(rT[:], lT[:])
            rrow = norm_pool.tile([1, HPC, W], FP32, tag="rrow")
            nc.gpsimd.dma_start(rrow[:], rT[:])
            bc = norm_pool.tile([D, HPC, W], FP32, tag="bc")
            nc.gpsimd.partition_broadcast(bc[:], rrow[:])
            qsl = slice(w * W, (w + 1) * W)
            for h in range(HPC):
                nc.vector.tensor_mul(
                    outTt[b][h * D : (h + 1) * D, qsl],
                    sg[0:D, h, :],
                    bc[:, h, :],
                )

        def proj_grp(b, g):
            """Output projection for t blocks 4g..4g+3 of batch b."""
            for tb in range(4 * g, 4 * g + 4):
                st = st_pool.tile([128, E], FP32, tag="st", name="st")
                for oc in range(2):
                    pj = mm_pool.tile([128, W], FP32, tag="mm", name="pj")
                    nc.tensor.matmul(
                        pj[:],
                        outTt[b][:, tb * 128 : (tb + 1) * 128],
                        wp_sb[:, oc * W : (oc + 1) * W],
                        start=True,
                        stop=True,
                    )
                    nc.vector.tensor_copy(st[:, oc * W : (oc + 1) * W], pj[:])
                nc.gpsimd.dma_start(outp[b, tb * 128 : (tb + 1) * 128, :], st[:])

        # --- emission: per-batch streams; the PV matmuls trail the
        # scores/exp stream by LEAD k-tiles ACROSS window seams so the
        # in-order PE queue never parks on a blocked PV (the previous
        # window's accumulator is released by one staging copy).  The
        # scheduler overlaps batch b+1's qkv / batch b-1's proj into the
        # ACT-bound attention span.
        LEAD = 6

        def batch_stream(b, extra):
            pending = []
            for w in range(NW):
                qkv_chunk(b, w)
                if b == 1:
                    extra(w)
                for kt in range(4 * w + 4):
                    pending.append((b, w, kt, *scores_exp(b, w, kt)))
                    if len(pending) > LEAD:
                        pv(*pending.pop(0))
            for ent in pending:
                pv(*ent)

        batch_stream(0, None)
        batch_stream(1, lambda c: proj_grp(0, c))

    nc.compile()
    return nc


def _get_nc():
    if not _NC_CACHE:
        _NC_CACHE.append(_build_nc())
    return _NC_CACHE[0]


def make_in_maps(x, w_qkv, w_proj):
    x = np.asarray(x, np.float32)
    w_qkv = np.asarray(w_qkv, np.float32)
    w_proj = np.asarray(w_proj, np.float32)
    xT = np.ascontiguousarray(x.transpose(2, 0, 1)).astype(NPBF)  # [E, B, T]
    in_maps = []
    for c in range(NCORES):
        h0 = c * HPC
        wq = w_qkv[h0 * D : (h0 + HPC) * D] * 0.125  # fold softmax scale
        wk = w_qkv[E + h0 * D : E + (h0 + HPC) * D]
        wv = w_qkv[2 * E + h0 * D : 2 * E + (h0 + HPC) * D]
        wqkvT = np.ascontiguousarray(np.concatenate([wq, wk, wv], 0).T)
        wpTc = np.ascontiguousarray(w_proj[:, c * JC : (c + 1) * JC].T)
        in_maps.append(
            {
                "xT": xT,
                "wqkvT": wqkvT.astype(NPBF),
                "wpT": wpTc.astype(NPBF),
            }
        )
    return in_maps


def kernel(x, w_qkv, w_proj, **run_kwargs):
    in_maps = make_in_maps(x, w_qkv, w_proj)
    nc = _get_nc()
    res = run_bass_kernel_spmd(nc, in_maps, core_ids=list(range(NCORES)), **run_kwargs)
    out = res.results[0]["outp"].copy()
    for r in res.results[1:]:
        out += r["outp"]
    if run_kwargs:
        kernel.last_results = res
    return out


# revision 28
# speedup vs baseline: 1.0437x; 1.0437x over previous
"""Causal self-attention (B=2, T=2048, E=1024, H=16) on 8 trn2 NeuronCores.

Sharding: tensor-parallel over heads - core c owns heads {2c, 2c+1}.
Each core computes qkv for its 2 heads, causal attention, and a partial
[B, T, E] output projection over its 128 columns of w_proj; the host
sums the 8 partials.

v2 design (ACT/PE balanced, ~all engines overlapped):
  - attention runs on 512-wide q windows; per k-tile the two heads'
    scores matmuls are a ROW-TILED CONCURRENT PAIR (K=64 contraction,
    h0 on PE rows 0-63, h1 on rows 64-127, auto tile_position from
    base_partition) writing one [128, 2, 512] PSUM tile (adjacent
    banks), so both heads' scores cost one matmul's wall time;
  - exp for BOTH heads is ONE ACTIVATE instruction (strided over the
    2-bank psum tile) - the scalar engine is the attention bottleneck
    (1 elem/cycle/lane), so instruction count is minimized;
  - qkv projection of batch b+1 and the output projection of batch b-1
    are emitted to interleave into the ACT-bound attention span, so the
    tensor engine never idles long enough to drop the HAM clock gate;
  - V^T tiles are produced by the DMA xbar transpose engine (not the
    PE) straight into the PV stationary layout;
  - softmax 1/l: l rows for both heads are contiguous in the [65,2,512]
    PV psum tile; one copy -> DMA-reshape to [128,8] -> exact DVE
    reciprocal -> DMA back -> gpsimd partition-broadcast -> 2 muls.

Matmul operands are bf16 (PSUM accumulation fp32); the softmax scale
1/8 is folded into w_q on the host.
"""

import numpy as np
import ml_dtypes
from contextlib import ExitStack

import concourse.bass as bass
import concourse.mybir as mybir
import concourse.tile as tile
from concourse import bacc
from concourse.bass_utils import run_bass_kernel_spmd

B, T, E, H, D = 2, 2048, 1024, 16, 64
NCORES = 8
HPC = H // NCORES          # heads per core = 2
JC = HPC * D               # local out-projection columns per core = 128
W = 512                    # q window (one PSUM bank of fp32)
KT = 128                   # k tile (matmul M limit)
NW = T // W                # windows per batch = 4
NKT = T // KT              # k tiles per batch = 16
NCH = 8                    # E/128 contraction chunks

BF16 = mybir.dt.bfloat16
FP32 = mybir.dt.float32
NPBF = ml_dtypes.bfloat16
EXP = mybir.ActivationFunctionType.Exp
CPY = mybir.ActivationFunctionType.Copy

_NC_CACHE = []


def _build_nc():
    nc = bacc.Bacc(None, target_bir_lowering=False)

    xT = nc.dram_tensor("xT", [E, B, T], BF16, kind="ExternalInput")
    wqkvT = nc.dram_tensor("wqkvT", [E, 3 * JC], BF16, kind="ExternalInput")
    wpT = nc.dram_tensor("wpT", [JC, E], BF16, kind="ExternalInput")
    outp = nc.dram_tensor("outp", [B, T, E], FP32, kind="ExternalOutput")

    with tile.TileContext(nc) as tc, ExitStack() as ctx:
        const_pool = ctx.enter_context(tc.tile_pool(name="const", bufs=1))
        w_pool = ctx.enter_context(tc.tile_pool(name="w", bufs=1))
        xt_pool = ctx.enter_context(tc.tile_pool(name="xt", bufs=1))
        qk_pool = ctx.enter_context(tc.tile_pool(name="qk", bufs=1))
        va_pool = ctx.enter_context(tc.tile_pool(name="va", bufs=1))
        vtmp_pool = ctx.enter_context(tc.tile_pool(name="vtmp", bufs=2))
        probs_pool = ctx.enter_context(tc.tile_pool(name="probs", bufs=8))
        outT_pool = ctx.enter_context(tc.tile_pool(name="outT", bufs=1))
        norm_pool = ctx.enter_context(tc.tile_pool(name="norm", bufs=2))
        st_pool = ctx.enter_context(tc.tile_pool(name="st", bufs=3))
        # PSUM: 8 banks = scp 2x2 + ops 2x1x... ops is [65,2,W] = 2 banks
        # single-buffered + mmbuf 2x1
        scp_pool = ctx.enter_context(tc.tile_pool(name="scp", bufs=2, space="PSUM"))
        ops_pool = ctx.enter_context(tc.tile_pool(name="ops", bufs=1, space="PSUM"))
        mm_pool = ctx.enter_context(tc.tile_pool(name="mmbuf", bufs=2, space="PSUM"))

        # --- constants -------------------------------------------------
        # warmup: pull the exp ACT_TABLE_LOAD (~2.7us) into the prologue
        wrm = const_pool.tile([1, 8], FP32, tag="wrm")
        nc.gpsimd.memset(wrm[:], 0.0)
        wrm2 = const_pool.tile([1, 8], BF16, tag="wrm2")
        nc.scalar.activation(wrm2[:], wrm[:], EXP)

        # mask2[p, h, j] = 1 iff j >= p (causal band for a diagonal tile),
        # same band replicated for both heads so one DVE mul covers both.
        mask2 = const_pool.tile([128, HPC, KT], BF16)
        nc.gpsimd.memset(mask2[:], 1.0)
        for h in range(HPC):
            nc.gpsimd.affine_select(
                out=mask2[:, h, :],
                in_=mask2[:, h, :],
                compare_op=mybir.AluOpType.is_ge,
                fill=0.0,
                base=0,
                channel_multiplier=-1,
                pattern=[[1, KT]],
            )

        # --- weights ---------------------------------------------------
        # weights + whole-row batch-1 x loads on the gpsimd SWDGE queue;
        # the SP hwdge queue is reserved for batch-0 x chunks (just in
        # time) + the xbar transposes so nothing big queues ahead of them
        wq_sb = [
            w_pool.tile([128, 3 * JC], BF16, tag=f"wq{i}", name=f"wq{i}")
            for i in range(NCH)
        ]
        for i in range(NCH):
            nc.gpsimd.dma_start(wq_sb[i][:], wqkvT[i * 128 : (i + 1) * 128, :])
        wp_sb = w_pool.tile([JC, E], BF16, tag="wp")
        nc.gpsimd.dma_start(wp_sb[:], wpT[:])

        xt = [[None] * NCH for _ in range(B)]
        for b in range(B):
            for i in range(NCH):
                xt[b][i] = xt_pool.tile(
                    [128, T], BF16, tag=f"xt{b}_{i}", name=f"xt{b}_{i}"
                )
        csl0 = slice(0, W)
        for i in range(NCH):
            nc.sync.dma_start(xt[0][i][:, csl0], xT[i * 128 : (i + 1) * 128, 0, csl0])
        for i in range(NCH):
            nc.gpsimd.dma_start(xt[1][i][:], xT[i * 128 : (i + 1) * 128, 1, :])

        # --- per-batch persistent tiles --------------------------------
        QT, KTs, vaug, outTt = [], [], [], []
        for b in range(B):
            QT.append(qk_pool.tile([128, T], BF16, tag=f"QT{b}", name=f"QT{b}"))
            KTs.append(qk_pool.tile([128, T], BF16, tag=f"KT{b}", name=f"KT{b}"))
            va = va_pool.tile(
                [128, NKT, HPC, D + 1], BF16, tag=f"va{b}", name=f"va{b}"
            )
            nc.gpsimd.memset(va[:, :, :, D : D + 1], 1.0)
            vaug.append(va)
            outTt.append(
                outT_pool.tile([128, T], BF16, tag=f"oT{b}", name=f"oT{b}")
            )

        def qkv_chunk(b, c):
            """Project x chunk c into Q^T/K^T cols [cW,(c+1)W) and V^T
            (via DMA xbar transpose) into vaug k-tiles 4c..4c+3."""
            csl = slice(c * W, (c + 1) * W)
            if b == 0 and c + 1 < NW:  # JIT prefetch of the next x chunk,
                # emitted FIRST so it queues ahead of this chunk's xbar
                # transposes on the in-order SP queue
                nsl = slice((c + 1) * W, (c + 2) * W)
                for i in range(NCH):
                    nc.sync.dma_start(
                        xt[0][i][:, nsl], xT[i * 128 : (i + 1) * 128, 0, nsl]
                    )
            for fb in (2, 0, 1):  # V first (transpose latency), K last
                pp = mm_pool.tile([128, W], FP32, tag="mm", name=f"pp{b}{c}{fb}")
                for ct in range(NCH):
                    nc.tensor.matmul(
                        pp[:],
                        wq_sb[ct][:, fb * 128 : (fb + 1) * 128],
                        xt[b][ct][:, csl],
                        start=(ct == 0),
                        stop=(ct == NCH - 1),
                    )
                if fb == 0:
                    nc.vector.tensor_copy(QT[b][:, csl], pp[:])
                elif fb == 1:
                    nc.vector.tensor_copy(KTs[b][:, csl], pp[:])
                else:
                    vtmp = vtmp_pool.tile([128, W], BF16, tag="vt", name=f"vt{b}{c}")
                    nc.vector.tensor_copy(vtmp[:], pp[:])
                    k0 = c * (W // KT)
                    for h in range(HPC):
                        # xbar transpose needs a contiguous dest; stage
                        # then copy into the ones-interleaved vaug slots
                        vstg = vtmp_pool.tile(
                            [128, W // KT, D], BF16, tag="vs", name=f"vs{b}{c}{h}"
                        )
                        nc.sync.dma_start_transpose(
                            vstg[:], vtmp[h * D : (h + 1) * D, :]
                        )
                        nc.vector.tensor_copy(
                            vaug[b][:, k0 : k0 + W // KT, h, 0:D], vstg[:]
                        )

        def scores_exp(b, w, kt):
            q0 = w * W
            koff = kt * KT - q0
            lo = max(0, koff)
            scp = scp_pool.tile([128, HPC, W], FP32, tag="scp", name="scp")
            ksl = slice(kt * KT, (kt + 1) * KT)
            for h in range(HPC):
                nc.tensor.matmul(
                    scp[:, h, lo:W],
                    KTs[b][h * D : (h + 1) * D, ksl],
                    QT[b][h * D : (h + 1) * D, q0 + lo : q0 + W],
                    start=True,
                    stop=True,
                )
            pr = probs_pool.tile([128, HPC, W], BF16, tag="pr", name="pr")
            nc.scalar.activation(pr[:, :, lo:W], scp[:, :, lo:W], EXP)
            if koff >= 0:  # diagonal tile: mask the 128-band per head
                for h in range(HPC):
                    nc.vector.tensor_mul(
                        pr[:, h, lo : lo + KT],
                        pr[:, h, lo : lo + KT],
                        mask2[:, h, :],
                    )
            return pr, lo

        ops_live = {}

        def pv(b, w, kt, pr, lo):
            nkt = 4 * w + 4
            if kt == 0:
                ops_live[b] = ops_pool.tile(
                    [D + 1, HPC, W], FP32, tag="ops", name=f"ops{b}{w}"
                )
            opsb = ops_live[b]
            for h in range(HPC):
                nc.tensor.matmul(
                    opsb[:, h, lo:W],
                    vaug[b][:, kt, h, :],
                    pr[:, h, lo:W],
                    start=(kt == 0),
                    stop=(kt == nkt - 1),
                    skip_group_check=True,
                )
            if kt == nkt - 1:
                finish_window(b, w, opsb)

        def finish_window(b, w, opsb):
            # Deferred normalize: ONE staging copy releases the psum
            # accumulator; the reciprocal/broadcast/mul chain then runs
            # entirely off the critical path from SBUF.
            sg = norm_pool.tile([D + 1, HPC, W], FP32, tag="sg", name=f"sg{b}{w}")
            nc.vector.tensor_copy(sg[:], opsb[0 : D + 1, :, :])
            lT = norm_pool.tile([128, HPC * W // 128], FP32, tag="lT")
            nc.gpsimd.dma_start(lT[:], sg[D : D + 1, :, :])
            rT = norm_pool.tile([128, HPC * W // 128], FP32, tag="rT")
            nc.vector.reciprocal(rT[:], lT[:])
            rrow = norm_pool.tile([1, HPC, W], FP32, tag="rrow")
            nc.gpsimd.dma_start(rrow[:], rT[:])
            bc = norm_pool.tile([D, HPC, W], FP32, tag="bc")
            nc.gpsimd.partition_broadcast(bc[:], rrow[:])
            qsl = slice(w * W, (w + 1) * W)
            for h in range(HPC):
                nc.vector.tensor_mul(
                    outTt[b][h * D : (h + 1) * D, qsl],
                    sg[0:D, h, :],
                    bc[:, h, :],
                )

        def proj_grp(b, g):
            """Output projection for t blocks 4g..4g+3 of batch b."""
            for tb in range(4 * g, 4 * g + 4):
                st = st_pool.tile([128, E], FP32, tag="st", name="st")
                for oc in range(2):
                    pj = mm_pool.tile([128, W], FP32, tag="mm", name="pj")
                    nc.tensor.matmul(
                        pj[:],
                        outTt[b][:, tb * 128 : (tb + 1) * 128],
                        wp_sb[:, oc * W : (oc + 1) * W],
                        start=True,
                        stop=True,
                    )
                    nc.vector.tensor_copy(st[:, oc * W : (oc + 1) * W], pj[:])
                nc.gpsimd.dma_start(outp[b, tb * 128 : (tb + 1) * 128, :], st[:])

        # --- emission: per-batch streams; the PV matmuls trail the
        # scores/exp stream by LEAD k-tiles ACROSS window seams so the
        # in-order PE queue never parks on a blocked PV (the previous
        # window's accumulator is released by one staging copy).  The
        # scheduler overlaps batch b+1's qkv / batch b-1's proj into the
        # ACT-bound attention span.
        LEAD = 5

        def batch_stream(b, extra):
            pending = []
            for w in range(NW):
                qkv_chunk(b, w)
                if b == 1:
                    extra(w)
                for kt in range(4 * w + 4):
                    pending.append((b, w, kt, *scores_exp(b, w, kt)))
                    if len(pending) > LEAD:
                        pv(*pending.pop(0))
            for ent in pending:
                pv(*ent)

        batch_stream(0, None)
        batch_stream(1, lambda c: proj_grp(0, c))
        for c in range(NW):
            proj_grp(1, c)

    nc.compile()
    return nc


def _get_nc():
    if not _NC_CACHE:
        _NC_CACHE.append(_build_nc())
    return _NC_CACHE[0]


def make_in_maps(x, w_qkv, w_proj):
    x = np.asarray(x, np.float32)
    w_qkv = np.asarray(w_qkv, np.float32)
    w_proj = np.asarray(w_proj, np.float32)
    xT = np.ascontiguousarray(x.transpose(2, 0, 1)).astype(NPBF)  # [E, B, T]
    in_maps = []
    for c in range(NCORES):
        h0 = c * HPC
        wq = w_qkv[h0 * D : (h0 + HPC) * D] * 0.125  # fold softmax scale
        wk = w_qkv[E + h0 * D : E + (h0 + HPC) * D]
        wv = w_qkv[2 * E + h0 * D : 2 * E + (h0 + HPC) * D]
        wqkvT = np.ascontiguousarray(np.concatenate([wq, wk, wv], 0).T)
        wpTc = np.ascontiguousarray(w_proj[:, c * JC : (c + 1) * JC].T)
        in_maps.append(
            {
                "xT": xT,
                "wqkvT": wqkvT.astype(NPBF),
                "wpT": wpTc.astype(NPBF),
            }
        )
    return in_maps


def kernel(x, w_qkv, w_proj, **run_kwargs):
    in_maps = make_in_maps(x, w_qkv, w_proj)
    nc = _get_nc()
    res = run_bass_kernel_spmd(nc, in_maps, core_ids=list(range(NCORES)), **run_kwargs)
    out = res.results[0]["outp"].copy()
    for r in res.results[1:]:
        out += r["outp"]
    if run_kwargs:
        kernel.last_results = res
    return out
